# revision 2
# baseline (speedup 1.0000x reference)
"""Trainium2 Bass kernel: per-(head,batch) euclidean compatibility matrix,
globally min/max-rescaled to [-9, 9].

reference (jax):
    q_sq = sum(Q*Q, -1)[..., :, None]
    k_sq = sum(K*K, -1)[..., None, :]
    cross = einsum("hbqd,hbgd->hbqg", Q, K)
    compat = sqrt(q_sq + k_sq - 2*cross)
    out = A_LO + (compat - min) * (A_HI - A_LO) / (max - min)   # min/max per (h,b)

Sharding: head h -> NeuronCore h (8 heads, 8 cores), fully independent.

Per-core program (B=4 slices of [N=2048, D=16]):
  - load Q[b]/K[b] in natural layout, augment each 128-row chunk with
    (ones, row-sum-of-squares) columns, PE-transpose to build
      UT = [Q^T; 1; q_sq]  (18 x 2048)     VT = [-2*K^T; k_sq; 1]  (18 x 2048)
    so that d2 = UT[:,q]^T @ VT[:,g] is the squared euclidean distance.
  - per 128-row q-tile: f32r matmul -> PSUM d2, ACT sqrt -> SBUF sq,
    fused DVE tensor_scalar+accum reduces (min / max partials).
  - finalize min/max across tiles + partitions (gpsimd all-reduce),
    compute c1 = 254/(max-min), c0 = 0.5 - min*c1 broadcast per partition.
  - per q-tile: u8 = round(sq*c1 + c0) in [0.5, 254.5] (DVE/ACT, RNE
    conversion), DMA u8 to DRAM.

Host I/O path (the axon tunnel moves ~65 MiB/s, so bytes dominate):
  - outputs quantized to u8 on device -> 128 MiB d2h instead of 512.
  - donated output buffers are created ON DEVICE (jit zeros) instead of
    shipping 512 MiB of host zeros up per call.
  - the jitted executable is cached across calls (the stock
    run_bass_kernel_spmd rebuilds + re-transfers everything per call).
  - per-shard fetch overlapped with host-side dequantization
    (out = u8 * 18/254 + (-9 - 9/254)), written into the final array.
"""

import concurrent.futures as _cf
import os
import time

import numpy as np

H, B, N, D = 8, 4, 2048, 16
A_LO, A_HI = -9.0, 9.0
P = 128
NT = N // P          # 16 q-tiles per slice
HALF = N // 2        # PSUM d2 tile width (2 banks)

# quantized affine target: min -> QLO, max -> QHI (stays inside u8 range
# so the RNE f32->u8 conversion can never wrap)
QLO, QHI = 0.5, 254.5
DEQ_S = (A_HI - A_LO) / (QHI - QLO)          # 18/254
DEQ_T = A_LO - QLO * DEQ_S                   # -9 - 0.5*18/254

# ---- tuning knobs ----
USE_F32R = True      # f32r matmul: 1 cyc/row vs 4 for plain f32
ACT_MADDS = 6        # of NT final-affine ops per slice routed to ACT (rest DVE)
SQ_BUFS = 17         # SBUF slots of [128, 2048] f32 for sq tiles
FUSED_REDUCE = True  # tensor_scalar+accum_out (2x mode) vs tensor_reduce (1x)

_CACHE = {}
_TIMING = bool(os.environ.get("KERNEL_TIMING"))


def _t(label, t0):
    if _TIMING:
        print(f"[kernel] {label}: {time.time()-t0:.3f}s", flush=True)
    return time.time()


def build_program():
    import concourse.bacc as bacc
    import concourse.bass as bass
    import concourse.mybir as mybir
    from concourse import tile, masks
    from concourse import bass_isa

    f32 = mybir.dt.float32
    u8 = mybir.dt.uint8
    Alu = mybir.AluOpType
    AF = mybir.ActivationFunctionType
    AX = mybir.AxisListType
    mmdt = mybir.dt.float32r if USE_F32R else f32

    nc = bacc.Bacc()
    Qd = nc.declare_dram_parameter("Q", [B, N, D], f32, isOutput=False)
    Kd = nc.declare_dram_parameter("K", [B, N, D], f32, isOutput=False)
    Od = nc.declare_dram_parameter("out", [B, N, N], u8, isOutput=True)

    with tile.TileContext(nc) as tc:
        with (
            tc.tile_pool(name="const", bufs=1) as constp,
            tc.tile_pool(name="ld", bufs=2) as ldp,
            tc.tile_pool(name="uv", bufs=2) as uvp,
            tc.tile_pool(name="sq", bufs=SQ_BUFS) as sqp,
            tc.tile_pool(name="u8o", bufs=4) as u8p,
            tc.tile_pool(name="dmy", bufs=2) as dmyp,
            tc.tile_pool(name="small", bufs=2) as smallp,
            tc.tile_pool(name="psd", bufs=3, space=bass.MemorySpace.PSUM) as psd,
            tc.tile_pool(name="pst", bufs=2, space=bass.MemorySpace.PSUM) as pst,
        ):
            ident = constp.tile([P, P], f32)
            masks.make_identity(nc, ident[:])

            for b in range(B):
                # ---------------- phase A: build UT / VT ----------------
                uts = []
                for (src, is_k) in ((Qd, False), (Kd, True)):
                    # cols 0:D = data, then for Q: col D = 1, col D+1 = q_sq
                    #                  for K: col D = k_sq, col D+1 = 1
                    # d2 = UT.T @ VT with VT = -2*[K^T; k_sq; 1] (copy scale -2)
                    # and UT = [Q^T; -1/2; -q_sq/2] (memset/TTR-scale -1/2).
                    ld = ldp.tile([P, NT, D + 2], f32, tag="ld")
                    nc.gpsimd.memset(ld[:], 1.0 if is_k else -0.5)
                    nc.sync.dma_start(
                        ld[:, :, 0:D], src[b].rearrange("(t p) d -> p t d", p=P)
                    )
                    sumcol = D if is_k else D + 1
                    TT = uvp.tile([D + 2, N], mmdt, tag="vt" if is_k else "ut")
                    for g in range(4):
                        ps = pst.tile([D + 2, 4 * P], f32, tag="tp")
                        for u in range(4):
                            t = g * 4 + u
                            # ld[:,t,sumcol] = scale * sum_d ld[:,t,d]^2
                            # (gpsimd square + DVE fused scale-sum; TTR
                            # with a broadcast out fails at runtime on HW)
                            sqld = dmyp.tile([P, D], f32, tag="sqld")
                            nc.gpsimd.tensor_tensor(
                                sqld[:], ld[:, t, 0:D], ld[:, t, 0:D], Alu.mult
                            )
                            dmy = dmyp.tile([P, 1], f32, tag="dmy")
                            nc.vector.tensor_scalar(
                                dmy[:].broadcast_to((P, D)),
                                sqld[:],
                                1.0 if is_k else -0.5,
                                None,
                                Alu.mult,
                                Alu.add,
                                accum_out=ld[:, t, sumcol : sumcol + 1],
                            )
                            nc.tensor.transpose(
                                ps[:, u * P : (u + 1) * P], ld[:, t, :], ident[:]
                            )
                        cols = slice(g * 4 * P, (g + 1) * 4 * P)
                        if is_k:
                            nc.scalar.mul(TT[:, cols], ps[:], -2.0)
                        else:
                            nc.scalar.copy(TT[:, cols], ps[:])
                    uts.append(TT)
                UT, VT = uts

                # ---------------- phase B: d2 -> sqrt -> min/max ----------------
                minp = smallp.tile([P, NT], f32, tag="minp")
                maxp = smallp.tile([P, NT], f32, tag="maxp")
                sqs = []
                for i in range(NT):
                    sq = sqp.tile([P, N], f32, tag="sq")
                    lhs = UT[:, i * P : (i + 1) * P]
                    for h in range(2):
                        d2 = psd.tile([P, HALF], f32, tag="d2")
                        for j in range(2):
                            c = h * 2 + j
                            nc.tensor.matmul(
                                d2[:, j * 512 : (j + 1) * 512],
                                lhs,
                                VT[:, c * 512 : (c + 1) * 512],
                                start=True,
                                stop=True,
                            )
                        nc.scalar.activation(
                            sq[:, h * HALF : (h + 1) * HALF], d2[:], AF.Sqrt
                        )
                    if FUSED_REDUCE:
                        # minp holds NEGATED per-tile minima (max of -sq), so
                        # both final reductions are max-reduces.
                        dm0 = dmyp.tile([P, 1], f32, tag="dmy")
                        nc.vector.tensor_scalar(
                            dm0[:].broadcast_to((P, N)),
                            sq[:],
                            -1.0,
                            None,
                            Alu.mult,
                            Alu.max,
                            accum_out=minp[:, i : i + 1],
                        )
                        dm1 = dmyp.tile([P, 1], f32, tag="dmy")
                        nc.vector.tensor_scalar(
                            dm1[:].broadcast_to((P, N)),
                            sq[:],
                            1.0,
                            None,
                            Alu.mult,
                            Alu.max,
                            accum_out=maxp[:, i : i + 1],
                        )
                    else:
                        nc.vector.tensor_reduce(
                            minp[:, i : i + 1], sq[:], AX.X, Alu.min
                        )
                        nc.vector.tensor_reduce(
                            maxp[:, i : i + 1], sq[:], AX.X, Alu.max
                        )
                    sqs.append(sq)

                # ---------------- phase C: finalize scalars ----------------
                # s[:,0] = -min (via negated partials), s[:,1] = max; one
                # gpsimd all-reduce handles both (both are max-reduces).
                s2 = smallp.tile([P, 2], f32, tag="s2")
                sr = smallp.tile([P, 2], f32, tag="sr")
                u = smallp.tile([P, 1], f32, tag="u")
                r = smallp.tile([P, 1], f32, tag="r")
                c1 = smallp.tile([P, 1], f32, tag="c1")
                t0 = smallp.tile([P, 1], f32, tag="t0")
                c0 = smallp.tile([P, 1], f32, tag="c0")

                if FUSED_REDUCE:
                    nc.vector.tensor_reduce(s2[:, 0:1], minp[:], AX.X, Alu.max)
                else:
                    m1 = smallp.tile([P, 1], f32, tag="m1")
                    nc.vector.tensor_reduce(m1[:], minp[:], AX.X, Alu.min)
                    nc.vector.tensor_scalar(
                        s2[:, 0:1], m1[:], -1.0, None, Alu.mult
                    )
                nc.vector.tensor_reduce(s2[:, 1:2], maxp[:], AX.X, Alu.max)
                nc.gpsimd.partition_all_reduce(
                    sr[:], s2[:], P, bass_isa.ReduceOp.max
                )
                nmn = sr[:, 0:1]  # -min, on every partition
                mx = sr[:, 1:2]  # max, on every partition
                # c1 = (QHI-QLO)/(mx-mn);  c0 = QLO - mn*c1 = QLO + nmn*c1
                nc.vector.tensor_tensor(u[:], mx, nmn, Alu.add)  # mx - mn
                nc.vector.reciprocal(r[:], u[:])
                nc.vector.tensor_scalar(c1[:], r[:], QHI - QLO, None, Alu.mult)
                nc.vector.tensor_tensor(t0[:], nmn, c1[:], Alu.mult)
                nc.vector.tensor_scalar(c0[:], t0[:], QLO, None, Alu.add)

                # ---------------- phase D: affine -> u8 + store ----------------
                for i in range(NT):
                    sq = sqs[i]
                    ot = u8p.tile([P, N], u8, tag="u8o")
                    if i % NT < ACT_MADDS:
                        nc.scalar.activation(
                            ot[:],
                            sq[:],
                            AF.Identity,
                            bias=c0[:, 0:1],
                            scale=c1[:, 0:1],
                        )
                    else:
                        nc.vector.tensor_scalar(
                            ot[:], sq[:], c1[:, 0:1], c0[:, 0:1], Alu.mult, Alu.add
                        )
                    nc.sync.dma_start(Od[b, i * P : (i + 1) * P, :], ot[:])

    nc.compile()
    return nc


def _get_exec():
    """Build (once) and cache the sharded jitted executable + helpers."""
    if "exec" in _CACHE:
        return _CACHE["exec"]

    import jax
    import jax.numpy as jnp
    from jax.experimental.shard_map import shard_map
    from jax.sharding import Mesh, NamedSharding, PartitionSpec

    from concourse import bass2jax, mybir

    nc = build_program()
    bass2jax.install_neuronx_cc_hook()

    assert nc.dbg_callbacks is None or not nc.dbg_callbacks, "dbg callbacks unsupported"
    partition_name = (
        nc.partition_id_tensor.name if nc.partition_id_tensor is not None else None
    )

    in_names, out_names, out_avals = [], [], []
    for alloc in nc.m.functions[0].allocations:
        if not isinstance(alloc, mybir.MemoryLocationSet):
            continue
        name = alloc.memorylocations[0].name
        if alloc.kind == "ExternalInput":
            if name != partition_name:
                in_names.append(name)
        elif alloc.kind == "ExternalOutput":
            out_names.append(name)
            out_avals.append(
                jax.core.ShapedArray(
                    tuple(alloc.tensor_shape), mybir.dt.np(alloc.dtype)
                )
            )
    # dbg_addr (if any) rides as an extra zero-filled input, like
    # run_bass_via_pjrt does.
    extra_in = {}
    if nc.dbg_addr is not None:
        extra_in[nc.dbg_addr.name] = np.zeros((1, 2), np.uint32)
    expected = {"Q", "K"} | set(extra_in)
    assert set(in_names) == expected, (in_names, expected)
    assert out_names == ["out"], out_names
    n_params = len(in_names)
    n_outs = len(out_names)
    all_in_names = in_names + out_names
    if partition_name is not None:
        all_in_names.append(partition_name)
    donate = tuple(range(n_params, n_params + n_outs))

    def _body(*args):
        operands = list(args)
        if partition_name is not None:
            operands.append(bass2jax.partition_id_tensor())
        outs = bass2jax._bass_exec_p.bind(
            *operands,
            out_avals=tuple(out_avals),
            in_names=tuple(all_in_names),
            out_names=tuple(out_names),
            lowering_input_output_aliases=(),
            sim_require_finite=True,
            sim_require_nnan=True,
            nc=nc,
        )
        return tuple(outs)

    devices = jax.devices()[:H]
    assert len(devices) == H, f"need {H} devices, have {len(jax.devices())}"
    mesh = Mesh(np.asarray(devices), ("core",))
    spec = PartitionSpec("core")
    sharded = jax.jit(
        shard_map(
            _body,
            mesh=mesh,
            in_specs=(spec,) * (n_params + n_outs),
            out_specs=(spec,) * n_outs,
            check_rep=False,
        ),
        donate_argnums=donate,
        keep_unused=True,
    )
    out_sh = NamedSharding(mesh, spec)
    zeros_fn = jax.jit(
        lambda: jnp.zeros((H * B, N, N), jnp.uint8), out_shardings=out_sh
    )
    in_sh = NamedSharding(mesh, spec)

    _CACHE["exec"] = (sharded, zeros_fn, in_names, extra_in, in_sh)
    return _CACHE["exec"]


def _dequant_into(u, dst):
    # dst = u * DEQ_S + DEQ_T  (u8 -> f32)
    np.multiply(u, np.float32(DEQ_S), out=dst, dtype=np.float32, casting="unsafe")
    np.add(dst, np.float32(DEQ_T), out=dst)


def kernel(**inputs) -> np.ndarray:
    import jax

    t0 = time.time()
    Q = np.ascontiguousarray(np.asarray(inputs["Q"], dtype=np.float32))
    K = np.ascontiguousarray(np.asarray(inputs["K"], dtype=np.float32))
    assert Q.shape == (H, B, N, D) and K.shape == (H, B, N, D)

    sharded, zeros_fn, in_names, extra_in, in_sh = _get_exec()
    t0 = _t("setup", t0)

    glob = {
        "Q": Q.reshape(H * B, N, D),
        "K": K.reshape(H * B, N, D),
    }
    for name, z in extra_in.items():
        glob[name] = np.broadcast_to(z, (H * z.shape[0], *z.shape[1:]))
    # start input h2d early (async), then create donated out bufs on device
    args = [jax.device_put(glob[name], in_sh) for name in in_names]
    zeros = zeros_fn()
    t0 = _t("h2d+zeros dispatch", t0)

    out = sharded(*args, zeros)[0]  # sharded u8 [H*B, N, N]
    out.block_until_ready()
    t0 = _t("exec", t0)

    # overlap per-shard d2h with host dequantization
    outf = np.empty((H, B, N, N), np.float32)
    shards = sorted(out.addressable_shards, key=lambda s: s.index[0].start)
    assert len(shards) == H
    with _cf.ThreadPoolExecutor(2) as ex:
        futs = []
        for h, s in enumerate(shards):
            u = np.asarray(s.data)  # 16 MiB d2h, blocking
            futs.append(ex.submit(_dequant_into, u, outf[h]))
        for f in futs:
            f.result()
    _t("d2h+dequant", t0)
    return outf


if __name__ == "__main__":
    # quick smoke: build only
    nc = build_program()
    print("build ok:", nc)


# revision 26
# speedup vs baseline: 5.6797x; 5.6797x over previous
"""Trainium2 Bass kernel: per-(head,batch) euclidean compatibility matrix,
globally min/max-rescaled to [-9, 9].

reference (jax):
    q_sq = sum(Q*Q, -1)[..., :, None]
    k_sq = sum(K*K, -1)[..., None, :]
    cross = einsum("hbqd,hbgd->hbqg", Q, K)
    compat = sqrt(q_sq + k_sq - 2*cross)
    out = A_LO + (compat - min) * (A_HI - A_LO) / (max - min)   # min/max per (h,b)

Sharding: head h -> NeuronCore h (8 heads, 8 cores), fully independent.

Per-core program (B=4 slices of [N=2048, D=16]):
  - load Q[b]/K[b] in natural layout, augment each 128-row chunk with
    (ones, row-sum-of-squares) columns, PE-transpose to build
      UT = [Q^T; 1; q_sq]  (18 x 2048)     VT = [-2*K^T; k_sq; 1]  (18 x 2048)
    so that d2 = UT[:,q]^T @ VT[:,g] is the squared euclidean distance.
  - per 128-row q-tile: f32r matmul -> PSUM d2, ACT sqrt -> SBUF sq,
    fused DVE tensor_scalar+accum reduces (min / max partials).
  - finalize min/max across tiles + partitions (gpsimd all-reduce),
    compute c1 = 126/(max-min), c0 = 0.5 - min*c1 broadcast per partition.
  - per q-tile: code = round(sq*c1 + c0) in [0.5, 126.5] (DVE/ACT, RNE
    f32->u8 conversion), then pack 8 consecutive 7-bit code PLANES into
    7 byte planes (DVE shifts + or), DMA packed bytes to DRAM.

Host I/O path (the axon relay moves ~50 MiB/s of incompressible data on
a single shared CPU core, so d2h bytes dominate everything):
  - outputs quantized to 7-bit on device -> 112 MiB d2h instead of 512
    (quant err 18/126/2 = 0.071 abs vs the 0.18 = 2e-2*9 tolerance).
  - f16 inputs halve the (incompressible) h2d bytes.
  - donated output buffers are created ON DEVICE (jit zeros) instead of
    shipping 512 MiB of host zeros up per call.
  - the jitted executable is cached across calls (the stock
    run_bass_kernel_spmd rebuilds + re-transfers everything per call).
  - execution is pipelined over B in BC-sized chunks: upload + exec of
    chunk c+1 queue behind the fetch of chunk c, so the fetch stream is
    the only serial cost; unpack+dequant runs in the fetch thread pool.
"""

import concurrent.futures as _cf
import os
import time

import numpy as np

H, B, N, D = 8, 4, 2048, 16
A_LO, A_HI = -9.0, 9.0
P = 128
NT = N // P          # 16 q-tiles per slice
HALF = N // 2        # PSUM d2 tile width (2 banks)

# quantized affine target: min -> QLO, max -> QHI (stays inside the code
# range so the RNE f32->u8 conversion can never wrap). With PACK7, codes
# are 7-bit (8 values packed into 7 bytes on device -> 12.5% fewer d2h
# bytes); quantization error 18/126/2 = 0.071 abs vs the 0.18 tolerance.
PACK7 = True
QHI_CODE = 126.5 if PACK7 else 254.5
QLO = 0.5
DEQ_S = (A_HI - A_LO) / (QHI_CODE - QLO)     # 18/126 (or 18/254 unpacked)
DEQ_T = A_LO - QLO * DEQ_S
G = N // 8                                   # pack groups per row
OW = G * 7 if PACK7 else N                   # output row bytes

# ---- tuning knobs ----
USE_F32R = True      # f32r matmul: 1 cyc/row vs 4 for plain f32
ACT_MADDS = 6        # of NT final-affine ops per slice routed to ACT (rest DVE)
SQ_BUFS = 17         # SBUF slots of [128, 2048] f32 for sq tiles
FUSED_REDUCE = True  # tensor_scalar+accum_out (2x mode) vs tensor_reduce (1x)
BC = 1               # b-slices per device call; B/BC calls pipelined so
                     # upload+exec of call c+1 hide behind the fetch of call c

_CACHE = {}
_TIMING = bool(os.environ.get("KERNEL_TIMING"))


def _t(label, t0):
    if _TIMING:
        print(f"[kernel] {label}: {time.time()-t0:.3f}s", flush=True)
    return time.time()


def build_program():
    import concourse.bacc as bacc
    import concourse.bass as bass
    import concourse.mybir as mybir
    from concourse import tile, masks
    from concourse import bass_isa

    f32 = mybir.dt.float32
    f16 = mybir.dt.float16
    u8 = mybir.dt.uint8
    Alu = mybir.AluOpType
    AF = mybir.ActivationFunctionType
    AX = mybir.AxisListType
    mmdt = mybir.dt.float32r if USE_F32R else f32

    nc = bacc.Bacc()
    # f16 inputs: halves the (incompressible) h2d bytes; adds ~3e-3 abs err
    Qd = nc.declare_dram_parameter("Q", [BC, N, D], f16, isOutput=False)
    Kd = nc.declare_dram_parameter("K", [BC, N, D], f16, isOutput=False)
    Od = nc.declare_dram_parameter("out", [BC, N, OW], u8, isOutput=True)

    with tile.TileContext(nc) as tc:
        with (
            tc.tile_pool(name="const", bufs=1) as constp,
            tc.tile_pool(name="ld", bufs=2) as ldp,
            tc.tile_pool(name="uv", bufs=2) as uvp,
            tc.tile_pool(name="sq", bufs=SQ_BUFS) as sqp,
            tc.tile_pool(name="u8o", bufs=4) as u8p,
            tc.tile_pool(name="pk", bufs=2) as pkp,
            tc.tile_pool(name="dmy", bufs=2) as dmyp,
            tc.tile_pool(name="small", bufs=2) as smallp,
            tc.tile_pool(name="psd", bufs=3, space=bass.MemorySpace.PSUM) as psd,
            tc.tile_pool(name="pst", bufs=2, space=bass.MemorySpace.PSUM) as pst,
        ):
            ident = constp.tile([P, P], f32)
            masks.make_identity(nc, ident[:])

            for b in range(BC):
                # ---------------- phase A: build UT / VT ----------------
                uts = []
                for (src, is_k) in ((Qd, False), (Kd, True)):
                    # cols 0:D = data, then for Q: col D = 1, col D+1 = q_sq
                    #                  for K: col D = k_sq, col D+1 = 1
                    # d2 = UT.T @ VT with VT = -2*[K^T; k_sq; 1] (copy scale -2)
                    # and UT = [Q^T; -1/2; -q_sq/2] (memset/TTR-scale -1/2).
                    st = ldp.tile([P, NT, D], f16, tag="st")
                    nc.sync.dma_start(
                        st[:], src[b].rearrange("(t p) d -> p t d", p=P)
                    )
                    ld = ldp.tile([P, NT, D + 2], f32, tag="ld")
                    nc.gpsimd.memset(ld[:], 1.0 if is_k else -0.5)
                    nc.vector.tensor_scalar(
                        ld[:, :, 0:D], st[:], 1.0, None, Alu.mult
                    )
                    sumcol = D if is_k else D + 1
                    TT = uvp.tile([D + 2, N], mmdt, tag="vt" if is_k else "ut")
                    for g in range(4):
                        ps = pst.tile([D + 2, 4 * P], f32, tag="tp")
                        for u in range(4):
                            t = g * 4 + u
                            # ld[:,t,sumcol] = scale * sum_d ld[:,t,d]^2
                            # (gpsimd square + DVE fused scale-sum; TTR
                            # with a broadcast out fails at runtime on HW)
                            sqld = dmyp.tile([P, D], f32, tag="sqld")
                            nc.gpsimd.tensor_tensor(
                                sqld[:], ld[:, t, 0:D], ld[:, t, 0:D], Alu.mult
                            )
                            dmy = dmyp.tile([P, 1], f32, tag="dmy")
                            nc.vector.tensor_scalar(
                                dmy[:].broadcast_to((P, D)),
                                sqld[:],
                                1.0 if is_k else -0.5,
                                None,
                                Alu.mult,
                                Alu.add,
                                accum_out=ld[:, t, sumcol : sumcol + 1],
                            )
                            nc.tensor.transpose(
                                ps[:, u * P : (u + 1) * P], ld[:, t, :], ident[:]
                            )
                        cols = slice(g * 4 * P, (g + 1) * 4 * P)
                        if is_k:
                            nc.scalar.mul(TT[:, cols], ps[:], -2.0)
                        else:
                            nc.scalar.copy(TT[:, cols], ps[:])
                    uts.append(TT)
                UT, VT = uts

                # ---------------- phase B: d2 -> sqrt -> min/max ----------------
                minp = smallp.tile([P, NT], f32, tag="minp")
                maxp = smallp.tile([P, NT], f32, tag="maxp")
                sqs = []
                for i in range(NT):
                    sq = sqp.tile([P, N], f32, tag="sq")
                    lhs = UT[:, i * P : (i + 1) * P]
                    for h in range(2):
                        d2 = psd.tile([P, HALF], f32, tag="d2")
                        for j in range(2):
                            c = h * 2 + j
                            nc.tensor.matmul(
                                d2[:, j * 512 : (j + 1) * 512],
                                lhs,
                                VT[:, c * 512 : (c + 1) * 512],
                                start=True,
                                stop=True,
                            )
                        nc.scalar.activation(
                            sq[:, h * HALF : (h + 1) * HALF], d2[:], AF.Sqrt
                        )
                    if FUSED_REDUCE:
                        # minp holds NEGATED per-tile minima (max of -sq), so
                        # both final reductions are max-reduces.
                        dm0 = dmyp.tile([P, 1], f32, tag="dmy")
                        nc.vector.tensor_scalar(
                            dm0[:].broadcast_to((P, N)),
                            sq[:],
                            -1.0,
                            None,
                            Alu.mult,
                            Alu.max,
                            accum_out=minp[:, i : i + 1],
                        )
                        dm1 = dmyp.tile([P, 1], f32, tag="dmy")
                        nc.vector.tensor_scalar(
                            dm1[:].broadcast_to((P, N)),
                            sq[:],
                            1.0,
                            None,
                            Alu.mult,
                            Alu.max,
                            accum_out=maxp[:, i : i + 1],
                        )
                    else:
                        nc.vector.tensor_reduce(
                            minp[:, i : i + 1], sq[:], AX.X, Alu.min
                        )
                        nc.vector.tensor_reduce(
                            maxp[:, i : i + 1], sq[:], AX.X, Alu.max
                        )
                    sqs.append(sq)

                # ---------------- phase C: finalize scalars ----------------
                # s[:,0] = -min (via negated partials), s[:,1] = max; one
                # gpsimd all-reduce handles both (both are max-reduces).
                s2 = smallp.tile([P, 2], f32, tag="s2")
                sr = smallp.tile([P, 2], f32, tag="sr")
                u = smallp.tile([P, 1], f32, tag="u")
                r = smallp.tile([P, 1], f32, tag="r")
                c1 = smallp.tile([P, 1], f32, tag="c1")
                t0 = smallp.tile([P, 1], f32, tag="t0")
                c0 = smallp.tile([P, 1], f32, tag="c0")

                if FUSED_REDUCE:
                    nc.vector.tensor_reduce(s2[:, 0:1], minp[:], AX.X, Alu.max)
                else:
                    m1 = smallp.tile([P, 1], f32, tag="m1")
                    nc.vector.tensor_reduce(m1[:], minp[:], AX.X, Alu.min)
                    nc.vector.tensor_scalar(
                        s2[:, 0:1], m1[:], -1.0, None, Alu.mult
                    )
                nc.vector.tensor_reduce(s2[:, 1:2], maxp[:], AX.X, Alu.max)
                nc.gpsimd.partition_all_reduce(
                    sr[:], s2[:], P, bass_isa.ReduceOp.max
                )
                nmn = sr[:, 0:1]  # -min, on every partition
                mx = sr[:, 1:2]  # max, on every partition
                # c1 = (QHI-QLO)/(mx-mn);  c0 = QLO - mn*c1 = QLO + nmn*c1
                nc.vector.tensor_tensor(u[:], mx, nmn, Alu.add)  # mx - mn
                nc.vector.reciprocal(r[:], u[:])
                nc.vector.tensor_scalar(c1[:], r[:], QHI_CODE - QLO, None, Alu.mult)
                nc.vector.tensor_tensor(t0[:], nmn, c1[:], Alu.mult)
                nc.vector.tensor_scalar(c0[:], t0[:], QLO, None, Alu.add)

                # ---------------- phase D: affine -> u8 (+pack) + store ---------
                for i in range(NT):
                    sq = sqs[i]
                    ot = u8p.tile([P, N], u8, tag="u8o")
                    if i % NT < ACT_MADDS:
                        nc.scalar.activation(
                            ot[:],
                            sq[:],
                            AF.Identity,
                            bias=c0[:, 0:1],
                            scale=c1[:, 0:1],
                        )
                    else:
                        nc.vector.tensor_scalar(
                            ot[:], sq[:], c1[:, 0:1], c0[:, 0:1], Alu.mult, Alu.add
                        )
                    if not PACK7:
                        nc.sync.dma_start(Od[b, i * P : (i + 1) * P, :], ot[:])
                        continue
                    # pack 8 7-bit code PLANES -> 7 byte planes; plane e
                    # holds columns e*G..(e+1)*G so every device op and the
                    # host unpack run on contiguous [*, G] runs:
                    #   bt[j] = (v[j] >> j) | (v[j+1] << (7-j))
                    vv = ot[:].rearrange("p (e g) -> p e g", g=G)
                    lo = pkp.tile([P, 7, G], u8, tag="lo")
                    hi = pkp.tile([P, 7, G], u8, tag="hi")
                    bt = pkp.tile([P, 7, G], u8, tag="bt")
                    for j in range(7):
                        nc.vector.tensor_scalar(
                            lo[:, j, :], vv[:, j, :], j, None,
                            Alu.logical_shift_right,
                        )
                        nc.vector.tensor_scalar(
                            hi[:, j, :], vv[:, j + 1, :], 7 - j, None,
                            Alu.logical_shift_left,
                        )
                    nc.vector.tensor_tensor(bt[:], lo[:], hi[:], Alu.bitwise_or)
                    nc.sync.dma_start(
                        Od[b, i * P : (i + 1) * P, :],
                        bt[:].rearrange("p e g -> p (e g)"),
                    )

    nc.compile()
    return nc


def _get_exec():
    """Build (once) and cache the sharded jitted executable + helpers."""
    if "exec" in _CACHE:
        return _CACHE["exec"]

    import jax
    import jax.numpy as jnp
    from jax.experimental.shard_map import shard_map
    from jax.sharding import Mesh, NamedSharding, PartitionSpec

    from concourse import bass2jax, mybir

    nc = build_program()
    bass2jax.install_neuronx_cc_hook()

    assert nc.dbg_callbacks is None or not nc.dbg_callbacks, "dbg callbacks unsupported"
    partition_name = (
        nc.partition_id_tensor.name if nc.partition_id_tensor is not None else None
    )

    in_names, out_names, out_avals = [], [], []
    for alloc in nc.m.functions[0].allocations:
        if not isinstance(alloc, mybir.MemoryLocationSet):
            continue
        name = alloc.memorylocations[0].name
        if alloc.kind == "ExternalInput":
            if name != partition_name:
                in_names.append(name)
        elif alloc.kind == "ExternalOutput":
            out_names.append(name)
            out_avals.append(
                jax.core.ShapedArray(
                    tuple(alloc.tensor_shape), mybir.dt.np(alloc.dtype)
                )
            )
    # dbg_addr (if any) rides as an extra zero-filled input, like
    # run_bass_via_pjrt does.
    extra_in = {}
    if nc.dbg_addr is not None:
        extra_in[nc.dbg_addr.name] = np.zeros((1, 2), np.uint32)
    expected = {"Q", "K"} | set(extra_in)
    assert set(in_names) == expected, (in_names, expected)
    assert out_names == ["out"], out_names
    n_params = len(in_names)
    n_outs = len(out_names)
    all_in_names = in_names + out_names
    if partition_name is not None:
        all_in_names.append(partition_name)
    donate = tuple(range(n_params, n_params + n_outs))

    def _body(*args):
        operands = list(args)
        if partition_name is not None:
            operands.append(bass2jax.partition_id_tensor())
        outs = bass2jax._bass_exec_p.bind(
            *operands,
            out_avals=tuple(out_avals),
            in_names=tuple(all_in_names),
            out_names=tuple(out_names),
            lowering_input_output_aliases=(),
            sim_require_finite=True,
            sim_require_nnan=True,
            nc=nc,
        )
        return tuple(outs)

    devices = jax.devices()[:H]
    assert len(devices) == H, f"need {H} devices, have {len(jax.devices())}"
    mesh = Mesh(np.asarray(devices), ("core",))
    spec = PartitionSpec("core")
    sharded = jax.jit(
        shard_map(
            _body,
            mesh=mesh,
            in_specs=(spec,) * (n_params + n_outs),
            out_specs=(spec,) * n_outs,
            check_rep=False,
        ),
        donate_argnums=donate,
        keep_unused=True,
    )
    out_sh = NamedSharding(mesh, spec)
    zeros_fn = jax.jit(
        lambda: jnp.zeros((H * BC, N, OW), jnp.uint8), out_shardings=out_sh
    )
    in_sh = NamedSharding(mesh, spec)

    _CACHE["exec"] = (sharded, zeros_fn, in_names, extra_in, in_sh)
    return _CACHE["exec"]


def _dequant_into(u, dst):
    # dst = u * DEQ_S + DEQ_T  (u8 codes -> f32)
    np.multiply(u, np.float32(DEQ_S), out=dst, dtype=np.float32, casting="unsafe")
    np.add(dst, np.float32(DEQ_T), out=dst)


def _unpack_dequant_into(u, dst):
    # u: [bc, N, 7*G] packed byte planes; dst: [bc, N, N] f32.
    # plane e holds columns e*G..(e+1)*G, so every op is contiguous and the
    # unpacked [8, G] planes are already in column order.
    bc = u.shape[0]
    b = u.reshape(bc, N, 7, G)
    v = np.empty((bc, N, 8, G), np.uint8)
    v[..., 0, :] = b[..., 0, :] & 0x7F
    for i in range(1, 7):
        v[..., i, :] = ((b[..., i - 1, :] >> (8 - i)) | (b[..., i, :] << i)) & 0x7F
    v[..., 7, :] = b[..., 6, :] >> 1
    _dequant_into(v.reshape(bc, N, N), dst)


def kernel(**inputs) -> np.ndarray:
    import jax

    t0 = time.time()
    Q = np.asarray(inputs["Q"])
    K = np.asarray(inputs["K"])
    assert Q.shape == (H, B, N, D) and K.shape == (H, B, N, D)

    sharded, zeros_fn, in_names, extra_in, in_sh = _get_exec()
    t0 = _t("setup", t0)

    Qh = Q.astype(np.float16)
    Kh = K.astype(np.float16)
    extra_glob = {
        name: np.broadcast_to(z, (H * z.shape[0], *z.shape[1:]))
        for name, z in extra_in.items()
    }
    n_chunks = B // BC

    def _chunk_glob(x, c):
        # [H, BC, N, D] slice -> global [H*BC, N, D] (view when BC == 1)
        if BC == 1:
            return x[:, c]
        return np.ascontiguousarray(x[:, c * BC : (c + 1) * BC]).reshape(
            H * BC, N, D
        )

    # dispatch everything asynchronously up front: uploads, donated output
    # buffers, and all chunk execs queue on the relay; the fetch loop below
    # then drains outputs in order while later chunks still execute.
    outs = []
    for c in range(n_chunks):
        glob = {"Q": _chunk_glob(Qh, c), "K": _chunk_glob(Kh, c), **extra_glob}
        args = [jax.device_put(glob[name], in_sh) for name in in_names]
        outs.append(sharded(*args, zeros_fn())[0])  # sharded u8 [H*BC, N, N]
    t0 = _t("dispatch", t0)

    # fetch all shards concurrently (the relay pipelines better with
    # parallel RPCs), dequantize each into its slice as it lands
    outf = np.empty((H, B, N, N), np.float32)

    deq = _unpack_dequant_into if PACK7 else _dequant_into

    def _fetch_dequant(task):
        c, h, s = task
        tf = time.time()
        u = np.asarray(s.data)  # [BC, N, OW] u8 d2h
        if _TIMING:
            print(
                f"[kernel]   chunk{c} shard{h} fetch: {time.time()-tf:.3f}s",
                flush=True,
            )
        deq(u, outf[h, c * BC : (c + 1) * BC])

    tasks = []
    for c, out in enumerate(outs):
        shards = sorted(out.addressable_shards, key=lambda s: s.index[0].start)
        assert len(shards) == H
        tasks.extend((c, h, s) for h, s in enumerate(shards))
    nthr = int(os.environ.get("KERNEL_FETCH_THREADS", "8"))
    with _cf.ThreadPoolExecutor(nthr) as ex:
        list(ex.map(_fetch_dequant, tasks))
    _t("d2h+dequant", t0)
    return outf


if __name__ == "__main__":
    # quick smoke: build only
    nc = build_program()
    print("build ok:", nc)
